# revision 5
# baseline (speedup 1.0000x reference)
"""Trainium2 Bass kernel for nn_AutocorrelationCorrelogram.

For nervegram [B=4, F=50, T=20000, C=2]: 300 periodic-Hann-windowed frames
of length 512 per (b,f,c) signal, circular autocorrelation via
Wiener-Khinchin (rfft -> |.|^2 -> irfft), relu, normalize by sqrt(zero
lag), keep 256 lags, mean over channels -> [4, 50, 300, 256].

Sharding: pure data parallel over the 200 (b,f) pairs -> 25 per core x 8
cores (SPMD, no collectives).

Kernel structure (per core, per superbatch of 20 frames x 25 bf):
  - DMA frames row-major [125 rows=(m,bf), 512t, 2c] (4KB contiguous rows)
  - PE-transpose to time-major yt[k] [128 t, 500 rows] per channel
  - rfft as matmuls with window folded into the DFT matrices; Wsin col 0
    carries the bin-256 cos column (sin col of bin 0 is identically zero)
  - P = Re^2 + Im^2 with row-0 fixups for the bin-256 trick
  - irfft matmuls use P *as the stationary operand* so the result lands
    as acf^T [rows, lags] (row-major for output DMA, per-partition norm);
    D is scaled by 0.25 so adding the two channels yields the channel
    mean of the normalized acf
  - norm: relu(acf * 1/sqrt(acf0 + 1e-30)) via ACT Sqrt + DVE reciprocal
    + ACT Relu with per-partition scale, then one DVE add for the
    channel mean.
"""

import sys

import numpy as np

sys.path.insert(0, "/opt/trn_rl_repo")

B, F, T, C = 4, 50, 20000, 2
NUM_FRAME = 300
LEN_FRAME = 512
LAGS = 256
NBINS = 257
N_CORES = 8
BF_PER_CORE = (B * F) // N_CORES  # 25

FRAMES_PER_SB = 20  # frames per superbatch
ROWS_PER_TILE = 125  # 5 frames x 25 bf
TILES_PER_SB = 4
N_SB_FULL = NUM_FRAME // FRAMES_PER_SB  # 15
NCOLS = 500  # rows per (c) group = 20*25

STARTS = np.linspace(0, T - LEN_FRAME, NUM_FRAME).astype(np.int64)


def build_weights():
    t = np.arange(LEN_FRAME, dtype=np.float64)
    w = 0.5 - 0.5 * np.cos(2.0 * np.pi * t / LEN_FRAME)  # periodic hann
    ang = 2.0 * np.pi * np.outer(t, np.arange(NBINS)) / LEN_FRAME
    Cm = np.cos(ang) * w[:, None]  # [512, 257]
    Sm = -np.sin(ang) * w[:, None]
    wcos = Cm[:, 0:256].reshape(4, 128, 256).copy()
    wsin = Sm[:, 0:256].reshape(4, 128, 256).copy()
    wsin[:, :, 0] = Cm[:, 256].reshape(4, 128)  # bin-256 cos column
    alpha = 0.25  # folds the channel-mean 0.5 (output scales with sqrt(alpha))
    k = np.arange(NBINS)
    coef = np.full(NBINS, 2.0)
    coef[0] = 1.0
    coef[256] = 1.0
    D = (alpha / LEN_FRAME) * coef[:, None] * np.cos(
        2.0 * np.pi * np.outer(k, np.arange(LAGS)) / LEN_FRAME
    )
    return (
        wcos.astype(np.float32),
        wsin.astype(np.float32),
        D.astype(np.float32),
        np.eye(128, dtype=np.float32),
    )


def build_nc(n_sb=N_SB_FULL, use_f32r=True):
    from contextlib import ExitStack

    import concourse.bacc as bacc
    import concourse.tile as tile
    from concourse import mybir

    f32 = mybir.dt.float32
    f32r = mybir.dt.float32r
    AF = mybir.ActivationFunctionType

    mmdt = f32r if use_f32r else f32

    nc = bacc.Bacc("TRN2", target_bir_lowering=False, debug=False)

    x = nc.dram_tensor("x", [BF_PER_CORE, T, C], f32, kind="ExternalInput").ap()
    wcos_d = nc.dram_tensor("wcos", [4, 128, 256], mmdt, kind="ExternalInput").ap()
    wsin_d = nc.dram_tensor("wsin", [4, 128, 256], mmdt, kind="ExternalInput").ap()
    dmat_d = nc.dram_tensor("dmat", [NBINS, LAGS], mmdt, kind="ExternalInput").ap()
    eye_d = nc.dram_tensor("eye", [128, 128], f32, kind="ExternalInput").ap()
    out = nc.dram_tensor(
        "out", [BF_PER_CORE, NUM_FRAME, LAGS], f32, kind="ExternalOutput"
    ).ap()

    with tile.TileContext(nc) as tc, ExitStack() as ctx:
        consts = ctx.enter_context(tc.tile_pool(name="consts", bufs=1))
        sb_pool = ctx.enter_context(tc.tile_pool(name="work", bufs=1))
        pp = ctx.enter_context(tc.tile_pool(name="ps", bufs=1, space="PSUM"))

        # ---- load constants once ----
        wcos_sb = consts.tile([128, 4, 256], mmdt, tag="wcos")
        wsin_sb = consts.tile([128, 4, 256], mmdt, tag="wsin")
        for k in range(4):
            nc.sync.dma_start(out=wcos_sb[:, k, :], in_=wcos_d[k])
            nc.sync.dma_start(out=wsin_sb[:, k, :], in_=wsin_d[k])
        dm0 = consts.tile([128, 256], mmdt, tag="dm0")
        dm1 = consts.tile([128, 256], mmdt, tag="dm1")
        dm2 = consts.tile([1, 256], mmdt, tag="dm2")
        nc.sync.dma_start(out=dm0[:], in_=dmat_d[0:128])
        nc.sync.dma_start(out=dm1[:], in_=dmat_d[128:256])
        nc.sync.dma_start(out=dm2[:], in_=dmat_d[256:257])
        eye_sb = consts.tile([128, 128], f32, tag="eye")
        nc.sync.dma_start(out=eye_sb[:], in_=eye_d[:])
        zero_b = consts.tile([128, 1], f32, tag="zerob")
        nc.vector.memset(zero_b[:], 0.0)
        eps_b = consts.tile([128, 1], f32, tag="epsb")
        nc.vector.memset(eps_b[:], 1e-30)

        for sb in range(n_sb):
            m0 = sb * FRAMES_PER_SB
            # ---- load frames: 4 tiles of [125 rows=(mm,bf), 512, 2] ----
            ftiles = []
            for j in range(TILES_PER_SB):
                ft = sb_pool.tile(
                    [ROWS_PER_TILE, LEN_FRAME, C], f32, tag="ft", bufs=8
                )
                for mm in range(5):
                    m = m0 + 5 * j + mm
                    s = int(STARTS[m])
                    nc.sync.dma_start(
                        out=ft[25 * mm : 25 * mm + 25],
                        in_=x[:, s : s + LEN_FRAME, :],
                    )
                ftiles.append(ft)

            norm_c0 = []
            for c in range(C):
                # ---- transpose to time-major yt[k] = [128 t, 500 rows] ----
                yts = []
                for k in range(4):
                    trp = pp.tile([128, NCOLS], f32, tag="tr", bufs=2)
                    for j in range(TILES_PER_SB):
                        nc.tensor.transpose(
                            trp[:, 125 * j : 125 * j + 125],
                            ftiles[j][:, 128 * k : 128 * k + 128, c : c + 1],
                            eye_sb[:125, :125],
                        )
                    yt = sb_pool.tile([128, NCOLS], mmdt, tag="yt", bufs=12)
                    nc.vector.tensor_copy(yt[:], trp[:])
                    yts.append(yt)

                # ---- rfft: Re/Im [128 bins, 500] per half ----
                re_ps, im_ps = [], []
                for h in range(2):
                    rp = pp.tile([128, NCOLS], f32, tag="fft", bufs=4)
                    ip = pp.tile([128, NCOLS], f32, tag="fft", bufs=4)
                    for k in range(4):
                        nc.tensor.matmul(
                            rp[:],
                            wcos_sb[:, k, 128 * h : 128 * h + 128],
                            yts[k][:],
                            start=(k == 0),
                            stop=(k == 3),
                        )
                        nc.tensor.matmul(
                            ip[:],
                            wsin_sb[:, k, 128 * h : 128 * h + 128],
                            yts[k][:],
                            start=(k == 0),
                            stop=(k == 3),
                        )
                    re_ps.append(rp)
                    im_ps.append(ip)

                # ---- P = Re^2 + Im^2 (+ bin-256 fixups) ----
                phs = []
                for h in range(2):
                    sq_r = sb_pool.tile([128, NCOLS], f32, tag="sqr", bufs=3)
                    sq_i = sb_pool.tile([128, NCOLS], f32, tag="sqi", bufs=3)
                    nc.scalar.activation(sq_r[:], re_ps[h][:], AF.Square, bias=zero_b[:])
                    nc.scalar.activation(sq_i[:], im_ps[h][:], AF.Square, bias=zero_b[:])
                    ph = sb_pool.tile([128, NCOLS], mmdt, tag=f"ph{h}", bufs=3)
                    nc.vector.tensor_add(ph[:], sq_r[:], sq_i[:])
                    phs.append(ph)
                p256 = sb_pool.tile([1, NCOLS], mmdt, tag="p256", bufs=3)
                # P256 = Im_h0[0]^2 ; fix P_h0[0] = Re_h0[0]^2
                nc.scalar.activation(p256[:], im_ps[0][0:1, :], AF.Square, bias=zero_b[0:1])
                nc.scalar.activation(phs[0][0:1, :], re_ps[0][0:1, :], AF.Square, bias=zero_b[0:1])

                # ---- irfft (P stationary) -> acf^T [125 rows, 256 lags] ----
                for g in range(4):
                    acfp = pp.tile([ROWS_PER_TILE, LAGS], f32, tag="acf", bufs=2)
                    sl = slice(125 * g, 125 * g + 125)
                    nc.tensor.matmul(
                        acfp[:], phs[0][:, sl], dm0[:],
                        start=True, stop=False,
                    )
                    nc.tensor.matmul(
                        acfp[:], phs[1][:, sl], dm1[:],
                        start=False, stop=False,
                    )
                    nc.tensor.matmul(
                        acfp[:], p256[:, sl], dm2[:],
                        start=False, stop=True,
                    )

                    # ---- normalize: relu(acf * rsqrt(acf0 + eps)) ----
                    sqc = sb_pool.tile([ROWS_PER_TILE, 1], f32, tag="sqc", bufs=8)
                    nc.scalar.activation(sqc[:], acfp[:, 0:1], AF.Sqrt, bias=eps_b[:125])
                    rcc = sb_pool.tile([ROWS_PER_TILE, 1], f32, tag="rcc", bufs=8)
                    nc.vector.reciprocal(out=rcc[:], in_=sqc[:])
                    nt = sb_pool.tile(
                        [ROWS_PER_TILE, LAGS], f32, tag=f"nt{c}",
                        bufs=(8 if c == 0 else 3),
                    )
                    nc.scalar.activation(nt[:], acfp[:], AF.Relu, bias=zero_b[:125], scale=rcc[:])
                    if c == 0:
                        norm_c0.append(nt)
                    else:
                        # ---- channel mean (0.5 folded into D) + store ----
                        mt = sb_pool.tile(
                            [ROWS_PER_TILE, LAGS], f32, tag="mt", bufs=6
                        )
                        nc.vector.tensor_add(mt[:], norm_c0[g][:], nt[:])
                        for mm in range(5):
                            m = m0 + 5 * g + mm
                            nc.sync.dma_start(
                                out=out[:, m, :],
                                in_=mt[25 * mm : 25 * mm + 25, :],
                            )

    nc.compile()
    return nc


_NC_CACHE = {}


def _get_nc(n_sb=N_SB_FULL, use_f32r=True):
    key = (n_sb, use_f32r)
    if key not in _NC_CACHE:
        _NC_CACHE[key] = build_nc(n_sb, use_f32r)
    return _NC_CACHE[key]


def make_in_maps(nerv):
    xs = nerv.reshape(B * F, T, C)
    wcos, wsin, dmat, eye = build_weights()
    return [
        {
            "x": np.ascontiguousarray(xs[BF_PER_CORE * i : BF_PER_CORE * (i + 1)]),
            "wcos": wcos,
            "wsin": wsin,
            "dmat": dmat,
            "eye": eye,
        }
        for i in range(N_CORES)
    ]


def kernel(nervegram, trace=False, use_f32r=True):
    from concourse.bass_utils import run_bass_kernel_spmd

    nerv = np.ascontiguousarray(np.asarray(nervegram, dtype=np.float32))
    assert nerv.shape == (B, F, T, C)
    in_maps = make_in_maps(nerv)
    nc = _get_nc(use_f32r=use_f32r)
    res = run_bass_kernel_spmd(nc, in_maps, list(range(N_CORES)), trace=trace)
    full = np.concatenate([res.results[i]["out"] for i in range(N_CORES)], axis=0)
    out = full.reshape(B, F, NUM_FRAME, LAGS)
    if trace:
        return out, res
    return out


# revision 7
# speedup vs baseline: 1.2561x; 1.2561x over previous
"""Trainium2 Bass kernel for nn_AutocorrelationCorrelogram.

For nervegram [B=4, F=50, T=20000, C=2]: 300 periodic-Hann-windowed frames
of length 512 per (b,f,c) signal, circular autocorrelation via
Wiener-Khinchin (rfft -> |.|^2 -> irfft), relu, normalize by sqrt(zero
lag), keep 256 lags, mean over channels -> [4, 50, 300, 256].

Sharding: pure data parallel over the 200 (b,f) pairs -> 25 per core x 8
cores (SPMD, no collectives).

Kernel structure (per core, per superbatch of 20 frames x 25 bf):
  - DMA frames row-major [125 rows=(m,bf), 512t, 2c] (4KB contiguous rows)
  - PE-transpose to time-major yt[k] [128 t, 500 rows] per channel
  - rfft as matmuls with window folded into the DFT matrices; Wsin col 0
    carries the bin-256 cos column (sin col of bin 0 is identically zero)
  - P = Re^2 + Im^2 with row-0 fixups for the bin-256 trick
  - irfft matmuls use P *as the stationary operand* so the result lands
    as acf^T [rows, lags] (row-major for output DMA, per-partition norm);
    D is scaled by 0.25 so adding the two channels yields the channel
    mean of the normalized acf
  - norm: relu(acf * 1/sqrt(acf0 + 1e-30)) via ACT Sqrt + DVE reciprocal
    + ACT Relu with per-partition scale, then one DVE add for the
    channel mean.
"""

import sys

import numpy as np

sys.path.insert(0, "/opt/trn_rl_repo")

B, F, T, C = 4, 50, 20000, 2
NUM_FRAME = 300
LEN_FRAME = 512
LAGS = 256
NBINS = 257
N_CORES = 8
BF_PER_CORE = (B * F) // N_CORES  # 25

FRAMES_PER_SB = 20  # frames per superbatch
ROWS_PER_TILE = 125  # 5 frames x 25 bf
TILES_PER_SB = 4
N_SB_FULL = NUM_FRAME // FRAMES_PER_SB  # 15
NCOLS = 500  # rows per (c) group = 20*25

STARTS = np.linspace(0, T - LEN_FRAME, NUM_FRAME).astype(np.int64)


def build_weights():
    t = np.arange(LEN_FRAME, dtype=np.float64)
    w = 0.5 - 0.5 * np.cos(2.0 * np.pi * t / LEN_FRAME)  # periodic hann
    ang = 2.0 * np.pi * np.outer(t, np.arange(NBINS)) / LEN_FRAME
    Cm = np.cos(ang) * w[:, None]  # [512, 257]
    Sm = -np.sin(ang) * w[:, None]
    wcos = Cm[:, 0:256].reshape(4, 128, 256).copy()
    wsin = Sm[:, 0:256].reshape(4, 128, 256).copy()
    wsin[:, :, 0] = Cm[:, 256].reshape(4, 128)  # bin-256 cos column
    alpha = 0.25  # folds the channel-mean 0.5 (output scales with sqrt(alpha))
    k = np.arange(NBINS)
    coef = np.full(NBINS, 2.0)
    coef[0] = 1.0
    coef[256] = 1.0
    D = (alpha / LEN_FRAME) * coef[:, None] * np.cos(
        2.0 * np.pi * np.outer(k, np.arange(LAGS)) / LEN_FRAME
    )
    return (
        wcos.astype(np.float32),
        wsin.astype(np.float32),
        D.astype(np.float32),
        np.eye(128, dtype=np.float32),
    )


def build_nc(n_sb=N_SB_FULL, use_f32r=True):
    from contextlib import ExitStack

    import concourse.bacc as bacc
    import concourse.bass as bass
    import concourse.tile as tile
    from concourse import mybir

    f32 = mybir.dt.float32
    f32r = mybir.dt.float32r
    AF = mybir.ActivationFunctionType

    mmdt = f32r if use_f32r else f32

    nc = bacc.Bacc("TRN2", target_bir_lowering=False, debug=False)

    x = nc.dram_tensor("x", [BF_PER_CORE, T, C], f32, kind="ExternalInput").ap()
    wcos_d = nc.dram_tensor("wcos", [4, 128, 256], mmdt, kind="ExternalInput").ap()
    wsin_d = nc.dram_tensor("wsin", [4, 128, 256], mmdt, kind="ExternalInput").ap()
    dmat_d = nc.dram_tensor("dmat", [NBINS, LAGS], mmdt, kind="ExternalInput").ap()
    eye_d = nc.dram_tensor("eye", [128, 128], f32, kind="ExternalInput").ap()
    out = nc.dram_tensor(
        "out", [BF_PER_CORE, NUM_FRAME, LAGS], f32, kind="ExternalOutput"
    ).ap()

    with tile.TileContext(nc) as tc, ExitStack() as ctx:
        consts = ctx.enter_context(tc.tile_pool(name="consts", bufs=1))
        sb_pool = ctx.enter_context(tc.tile_pool(name="work", bufs=1))
        pp = ctx.enter_context(tc.tile_pool(name="ps", bufs=1, space="PSUM"))

        # ---- load constants once ----
        wcos_sb = consts.tile([128, 4, 256], mmdt, tag="wcos")
        wsin_sb = consts.tile([128, 4, 256], mmdt, tag="wsin")
        for k in range(4):
            nc.sync.dma_start(out=wcos_sb[:, k, :], in_=wcos_d[k])
            nc.sync.dma_start(out=wsin_sb[:, k, :], in_=wsin_d[k])
        dm0 = consts.tile([128, 256], mmdt, tag="dm0")
        dm1 = consts.tile([128, 256], mmdt, tag="dm1")
        dm2 = consts.tile([1, 256], mmdt, tag="dm2")
        nc.sync.dma_start(out=dm0[:], in_=dmat_d[0:128])
        nc.sync.dma_start(out=dm1[:], in_=dmat_d[128:256])
        nc.sync.dma_start(out=dm2[:], in_=dmat_d[256:257])
        eye_sb = consts.tile([128, 128], f32, tag="eye")
        nc.sync.dma_start(out=eye_sb[:], in_=eye_d[:])
        zero_b = consts.tile([128, 1], f32, tag="zerob")
        nc.vector.memset(zero_b[:], 0.0)
        eps_b = consts.tile([128, 1], f32, tag="epsb")
        nc.vector.memset(eps_b[:], 1e-30)

        for sb in range(n_sb):
            m0 = sb * FRAMES_PER_SB
            # ---- load frames: 4 tiles of [125 rows=(mm,bf), 512, 2] ----
            ftiles = []
            for j in range(TILES_PER_SB):
                ft = sb_pool.tile(
                    [ROWS_PER_TILE, LEN_FRAME, C], f32, tag="ft", bufs=8
                )
                # fold frames with equal start-stride into one DMA
                mm = 0
                while mm < 5:
                    m = m0 + 5 * j + mm
                    run = 1
                    while (
                        mm + run < 5
                        and STARTS[m + run] - STARTS[m + run - 1]
                        == STARTS[m + 1] - STARTS[m]
                    ):
                        run += 1
                    s = int(STARTS[m])
                    step = int(STARTS[m + 1] - STARTS[m]) if run > 1 else 0
                    src_ap = bass.AP(
                        tensor=x.tensor,
                        offset=x.offset + s * C,
                        ap=[
                            [step * C, run],
                            [T * C, BF_PER_CORE],
                            [C, LEN_FRAME],
                            [1, C],
                        ],
                    )
                    nc.sync.dma_start(
                        out=ft[25 * mm : 25 * (mm + run)], in_=src_ap
                    )
                    mm += run
                ftiles.append(ft)

            norm_c0 = []
            for c in range(C):
                # ---- transpose to time-major yt[k] = [128 t, 500 rows] ----
                yts = []
                for k in range(4):
                    trp = pp.tile([128, NCOLS], f32, tag="tr", bufs=2)
                    for j in range(TILES_PER_SB):
                        nc.tensor.transpose(
                            trp[:, 125 * j : 125 * j + 125],
                            ftiles[j][:, 128 * k : 128 * k + 128, c : c + 1],
                            eye_sb[:125, :125],
                        )
                    yt = sb_pool.tile([128, NCOLS], mmdt, tag="yt", bufs=12)
                    nc.vector.tensor_copy(yt[:], trp[:])
                    yts.append(yt)

                # ---- rfft: Re/Im [128 bins, 500] per half ----
                re_ps, im_ps = [], []
                for h in range(2):
                    rp = pp.tile([128, NCOLS], f32, tag="fft", bufs=4)
                    ip = pp.tile([128, NCOLS], f32, tag="fft", bufs=4)
                    for k in range(4):
                        nc.tensor.matmul(
                            rp[:],
                            wcos_sb[:, k, 128 * h : 128 * h + 128],
                            yts[k][:],
                            start=(k == 0),
                            stop=(k == 3),
                        )
                        nc.tensor.matmul(
                            ip[:],
                            wsin_sb[:, k, 128 * h : 128 * h + 128],
                            yts[k][:],
                            start=(k == 0),
                            stop=(k == 3),
                        )
                    re_ps.append(rp)
                    im_ps.append(ip)

                # ---- P = Re^2 + Im^2 (+ bin-256 fixups) ----
                phs = []
                for h in range(2):
                    sq_r = sb_pool.tile([128, NCOLS], f32, tag="sqr", bufs=3)
                    sq_i = sb_pool.tile([128, NCOLS], f32, tag="sqi", bufs=3)
                    nc.scalar.activation(sq_r[:], re_ps[h][:], AF.Square, bias=zero_b[:])
                    nc.scalar.activation(sq_i[:], im_ps[h][:], AF.Square, bias=zero_b[:])
                    ph = sb_pool.tile([128, NCOLS], mmdt, tag=f"ph{h}", bufs=3)
                    nc.vector.tensor_add(ph[:], sq_r[:], sq_i[:])
                    phs.append(ph)
                p256 = sb_pool.tile([1, NCOLS], mmdt, tag="p256", bufs=3)
                # P256 = Im_h0[0]^2 ; fix P_h0[0] = Re_h0[0]^2
                nc.scalar.activation(p256[:], im_ps[0][0:1, :], AF.Square, bias=zero_b[0:1])
                nc.scalar.activation(phs[0][0:1, :], re_ps[0][0:1, :], AF.Square, bias=zero_b[0:1])

                # ---- irfft (P stationary) -> acf^T [125 rows, 256 lags] ----
                for g in range(4):
                    acfp = pp.tile([ROWS_PER_TILE, LAGS], f32, tag="acf", bufs=2)
                    sl = slice(125 * g, 125 * g + 125)
                    nc.tensor.matmul(
                        acfp[:], phs[0][:, sl], dm0[:],
                        start=True, stop=False,
                    )
                    nc.tensor.matmul(
                        acfp[:], phs[1][:, sl], dm1[:],
                        start=False, stop=False,
                    )
                    nc.tensor.matmul(
                        acfp[:], p256[:, sl], dm2[:],
                        start=False, stop=True,
                    )

                    # ---- normalize: relu(acf * rsqrt(acf0 + eps)) ----
                    sqc = sb_pool.tile([ROWS_PER_TILE, 1], f32, tag="sqc", bufs=8)
                    nc.scalar.activation(sqc[:], acfp[:, 0:1], AF.Sqrt, bias=eps_b[:125])
                    rcc = sb_pool.tile([ROWS_PER_TILE, 1], f32, tag="rcc", bufs=8)
                    nc.vector.reciprocal(out=rcc[:], in_=sqc[:])
                    nt = sb_pool.tile(
                        [ROWS_PER_TILE, LAGS], f32, tag=f"nt{c}",
                        bufs=(8 if c == 0 else 3),
                    )
                    nc.scalar.activation(nt[:], acfp[:], AF.Relu, bias=zero_b[:125], scale=rcc[:])
                    if c == 0:
                        norm_c0.append(nt)
                    else:
                        # ---- channel mean (0.5 folded into D) + store ----
                        mt = sb_pool.tile(
                            [ROWS_PER_TILE, LAGS], f32, tag="mt", bufs=6
                        )
                        nc.vector.tensor_add(mt[:], norm_c0[g][:], nt[:])
                        mf = m0 + 5 * g
                        nc.gpsimd.dma_start(
                            out=out[:, mf : mf + 5, :].rearrange(
                                "bf mm l -> mm bf l"
                            ),
                            in_=mt[:],
                        )

    nc.compile()
    return nc


_NC_CACHE = {}


def _get_nc(n_sb=N_SB_FULL, use_f32r=True):
    key = (n_sb, use_f32r)
    if key not in _NC_CACHE:
        _NC_CACHE[key] = build_nc(n_sb, use_f32r)
    return _NC_CACHE[key]


def make_in_maps(nerv):
    xs = nerv.reshape(B * F, T, C)
    wcos, wsin, dmat, eye = build_weights()
    return [
        {
            "x": np.ascontiguousarray(xs[BF_PER_CORE * i : BF_PER_CORE * (i + 1)]),
            "wcos": wcos,
            "wsin": wsin,
            "dmat": dmat,
            "eye": eye,
        }
        for i in range(N_CORES)
    ]


def kernel(nervegram, trace=False, use_f32r=True):
    from concourse.bass_utils import run_bass_kernel_spmd

    nerv = np.ascontiguousarray(np.asarray(nervegram, dtype=np.float32))
    assert nerv.shape == (B, F, T, C)
    in_maps = make_in_maps(nerv)
    nc = _get_nc(use_f32r=use_f32r)
    res = run_bass_kernel_spmd(nc, in_maps, list(range(N_CORES)), trace=trace)
    full = np.concatenate([res.results[i]["out"] for i in range(N_CORES)], axis=0)
    out = full.reshape(B, F, NUM_FRAME, LAGS)
    if trace:
        return out, res
    return out


# revision 8
# speedup vs baseline: 1.6776x; 1.3356x over previous
"""Trainium2 Bass kernel for nn_AutocorrelationCorrelogram.

For nervegram [B=4, F=50, T=20000, C=2]: 300 periodic-Hann-windowed frames
of length 512 per (b,f,c) signal, circular autocorrelation via
Wiener-Khinchin (rfft -> |.|^2 -> irfft), relu, normalize by sqrt(zero
lag), keep 256 lags, mean over channels -> [4, 50, 300, 256].

Sharding: pure data parallel over the 200 (b,f) pairs -> 25 per core x 8
cores (SPMD, no collectives).

Kernel structure (per core, per superbatch of 20 frames x 25 bf):
  - DMA frames row-major [125 rows=(m,bf), 512t, 2c] (4KB contiguous rows)
  - PE-transpose to time-major yt[k] [128 t, 500 rows] per channel
  - rfft as matmuls with window folded into the DFT matrices; Wsin col 0
    carries the bin-256 cos column (sin col of bin 0 is identically zero)
  - P = Re^2 + Im^2 with row-0 fixups for the bin-256 trick
  - irfft matmuls use P *as the stationary operand* so the result lands
    as acf^T [rows, lags] (row-major for output DMA, per-partition norm);
    D is scaled by 0.25 so adding the two channels yields the channel
    mean of the normalized acf
  - norm: relu(acf * 1/sqrt(acf0 + 1e-30)) via ACT Sqrt + DVE reciprocal
    + ACT Relu with per-partition scale, then one DVE add for the
    channel mean.
"""

import sys

import numpy as np

sys.path.insert(0, "/opt/trn_rl_repo")

B, F, T, C = 4, 50, 20000, 2
NUM_FRAME = 300
LEN_FRAME = 512
LAGS = 256
NBINS = 257
N_CORES = 8
BF_PER_CORE = (B * F) // N_CORES  # 25

FRAMES_PER_SB = 20  # frames per superbatch
ROWS_PER_TILE = 125  # 5 frames x 25 bf
TILES_PER_SB = 4
N_SB_FULL = NUM_FRAME // FRAMES_PER_SB  # 15
NCOLS = 500  # rows per (c) group = 20*25

STARTS = np.linspace(0, T - LEN_FRAME, NUM_FRAME).astype(np.int64)


def build_weights():
    t = np.arange(LEN_FRAME, dtype=np.float64)
    w = 0.5 - 0.5 * np.cos(2.0 * np.pi * t / LEN_FRAME)  # periodic hann
    ang = 2.0 * np.pi * np.outer(t, np.arange(NBINS)) / LEN_FRAME
    Cm = np.cos(ang) * w[:, None]  # [512, 257]
    Sm = -np.sin(ang) * w[:, None]
    wcos = Cm[:, 0:256].reshape(4, 128, 256).copy()
    wsin = Sm[:, 0:256].reshape(4, 128, 256).copy()
    wsin[:, :, 0] = Cm[:, 256].reshape(4, 128)  # bin-256 cos column
    alpha = 0.25  # folds the channel-mean 0.5 (output scales with sqrt(alpha))
    k = np.arange(NBINS)
    coef = np.full(NBINS, 2.0)
    coef[0] = 1.0
    coef[256] = 1.0
    D = (alpha / LEN_FRAME) * coef[:, None] * np.cos(
        2.0 * np.pi * np.outer(k, np.arange(LAGS)) / LEN_FRAME
    )
    return (
        wcos.astype(np.float32),
        wsin.astype(np.float32),
        D.astype(np.float32),
        np.eye(128, dtype=np.float32),
    )


def build_nc(n_sb=N_SB_FULL, use_f32r=True):
    from contextlib import ExitStack

    import concourse.bacc as bacc
    import concourse.bass as bass
    import concourse.tile as tile
    from concourse import mybir

    f32 = mybir.dt.float32
    f32r = mybir.dt.float32r
    AF = mybir.ActivationFunctionType

    mmdt = f32r if use_f32r else f32

    nc = bacc.Bacc("TRN2", target_bir_lowering=False, debug=False)

    x = nc.dram_tensor("x", [BF_PER_CORE, T, C], f32, kind="ExternalInput").ap()
    wcos_d = nc.dram_tensor("wcos", [4, 128, 256], mmdt, kind="ExternalInput").ap()
    wsin_d = nc.dram_tensor("wsin", [4, 128, 256], mmdt, kind="ExternalInput").ap()
    dmat_d = nc.dram_tensor("dmat", [NBINS, LAGS], mmdt, kind="ExternalInput").ap()
    eye_d = nc.dram_tensor("eye", [128, 128], f32, kind="ExternalInput").ap()
    out = nc.dram_tensor(
        "out", [BF_PER_CORE, NUM_FRAME, LAGS], f32, kind="ExternalOutput"
    ).ap()

    with tile.TileContext(nc) as tc, ExitStack() as ctx:
        consts = ctx.enter_context(tc.tile_pool(name="consts", bufs=1))
        sb_pool = ctx.enter_context(tc.tile_pool(name="work", bufs=1))
        pp = ctx.enter_context(tc.tile_pool(name="ps", bufs=1, space="PSUM"))

        # ---- load constants once ----
        wcos_sb = consts.tile([128, 4, 256], mmdt, tag="wcos")
        wsin_sb = consts.tile([128, 4, 256], mmdt, tag="wsin")
        for k in range(4):
            nc.sync.dma_start(out=wcos_sb[:, k, :], in_=wcos_d[k])
            nc.sync.dma_start(out=wsin_sb[:, k, :], in_=wsin_d[k])
        dm0 = consts.tile([128, 256], mmdt, tag="dm0")
        dm1 = consts.tile([128, 256], mmdt, tag="dm1")
        dm2 = consts.tile([1, 256], mmdt, tag="dm2")
        nc.sync.dma_start(out=dm0[:], in_=dmat_d[0:128])
        nc.sync.dma_start(out=dm1[:], in_=dmat_d[128:256])
        nc.sync.dma_start(out=dm2[:], in_=dmat_d[256:257])
        eye_sb = consts.tile([128, 128], f32, tag="eye")
        nc.sync.dma_start(out=eye_sb[:], in_=eye_d[:])
        zero_b = consts.tile([128, 1], f32, tag="zerob")
        nc.vector.memset(zero_b[:], 0.0)
        eps_b = consts.tile([128, 1], f32, tag="epsb")
        nc.vector.memset(eps_b[:], 1e-30)

        for sb in range(n_sb):
            m0 = sb * FRAMES_PER_SB
            # ---- load frames: 4 tiles of [125 rows=(mm,bf), 512, 2] ----
            ftiles = []
            for j in range(TILES_PER_SB):
                ft = sb_pool.tile(
                    [ROWS_PER_TILE, LEN_FRAME, C], f32, tag="ft", bufs=8
                )
                # fold frames with equal start-stride into one DMA
                mm = 0
                while mm < 5:
                    m = m0 + 5 * j + mm
                    run = 1
                    while (
                        mm + run < 5
                        and STARTS[m + run] - STARTS[m + run - 1]
                        == STARTS[m + 1] - STARTS[m]
                    ):
                        run += 1
                    s = int(STARTS[m])
                    step = int(STARTS[m + 1] - STARTS[m]) if run > 1 else 0
                    src_ap = bass.AP(
                        tensor=x.tensor,
                        offset=x.offset + s * C,
                        ap=[
                            [step * C, run],
                            [T * C, BF_PER_CORE],
                            [C, LEN_FRAME],
                            [1, C],
                        ],
                    )
                    nc.gpsimd.dma_start(
                        out=ft[25 * mm : 25 * (mm + run)], in_=src_ap
                    )
                    mm += run
                ftiles.append(ft)

            norm_c0 = []
            for c in range(C):
                # ---- transpose to time-major yt[k] = [128 t, 500 rows] ----
                yts = []
                for k in range(4):
                    trp = pp.tile([128, NCOLS], f32, tag="tr", bufs=2)
                    for j in range(TILES_PER_SB):
                        nc.tensor.transpose(
                            trp[:, 125 * j : 125 * j + 125],
                            ftiles[j][:, 128 * k : 128 * k + 128, c : c + 1],
                            eye_sb[:125, :125],
                        )
                    yt = sb_pool.tile([128, NCOLS], mmdt, tag="yt", bufs=12)
                    nc.vector.tensor_copy(yt[:], trp[:])
                    yts.append(yt)

                # ---- rfft: Re/Im [128 bins, 500] per half ----
                re_ps, im_ps = [], []
                for h in range(2):
                    rp = pp.tile([128, NCOLS], f32, tag="fft", bufs=4)
                    ip = pp.tile([128, NCOLS], f32, tag="fft", bufs=4)
                    for k in range(4):
                        nc.tensor.matmul(
                            rp[:],
                            wcos_sb[:, k, 128 * h : 128 * h + 128],
                            yts[k][:],
                            start=(k == 0),
                            stop=(k == 3),
                        )
                        nc.tensor.matmul(
                            ip[:],
                            wsin_sb[:, k, 128 * h : 128 * h + 128],
                            yts[k][:],
                            start=(k == 0),
                            stop=(k == 3),
                        )
                    re_ps.append(rp)
                    im_ps.append(ip)

                # ---- P = Re^2 + Im^2 (+ bin-256 fixups) ----
                phs = []
                for h in range(2):
                    sq_r = sb_pool.tile([128, NCOLS], f32, tag="sqr", bufs=3)
                    sq_i = sb_pool.tile([128, NCOLS], f32, tag="sqi", bufs=3)
                    nc.scalar.activation(sq_r[:], re_ps[h][:], AF.Square, bias=zero_b[:])
                    nc.scalar.activation(sq_i[:], im_ps[h][:], AF.Square, bias=zero_b[:])
                    ph = sb_pool.tile([128, NCOLS], mmdt, tag=f"ph{h}", bufs=3)
                    nc.vector.tensor_add(ph[:], sq_r[:], sq_i[:])
                    phs.append(ph)
                p256 = sb_pool.tile([1, NCOLS], mmdt, tag="p256", bufs=3)
                # P256 = Im_h0[0]^2 ; fix P_h0[0] = Re_h0[0]^2
                nc.scalar.activation(p256[:], im_ps[0][0:1, :], AF.Square, bias=zero_b[0:1])
                nc.scalar.activation(phs[0][0:1, :], re_ps[0][0:1, :], AF.Square, bias=zero_b[0:1])

                # ---- irfft (P stationary) -> acf^T [125 rows, 256 lags] ----
                for g in range(4):
                    acfp = pp.tile([ROWS_PER_TILE, LAGS], f32, tag="acf", bufs=2)
                    sl = slice(125 * g, 125 * g + 125)
                    nc.tensor.matmul(
                        acfp[:], phs[0][:, sl], dm0[:],
                        start=True, stop=False,
                    )
                    nc.tensor.matmul(
                        acfp[:], phs[1][:, sl], dm1[:],
                        start=False, stop=False,
                    )
                    nc.tensor.matmul(
                        acfp[:], p256[:, sl], dm2[:],
                        start=False, stop=True,
                    )

                    # ---- normalize: relu(acf * rsqrt(acf0 + eps)) ----
                    sqc = sb_pool.tile([ROWS_PER_TILE, 1], f32, tag="sqc", bufs=8)
                    nc.scalar.activation(sqc[:], acfp[:, 0:1], AF.Sqrt, bias=eps_b[:125])
                    rcc = sb_pool.tile([ROWS_PER_TILE, 1], f32, tag="rcc", bufs=8)
                    nc.vector.reciprocal(out=rcc[:], in_=sqc[:])
                    nt = sb_pool.tile(
                        [ROWS_PER_TILE, LAGS], f32, tag=f"nt{c}",
                        bufs=(8 if c == 0 else 3),
                    )
                    nc.scalar.activation(nt[:], acfp[:], AF.Relu, bias=zero_b[:125], scale=rcc[:])
                    if c == 0:
                        norm_c0.append(nt)
                    else:
                        # ---- channel mean (0.5 folded into D) + store ----
                        mt = sb_pool.tile(
                            [ROWS_PER_TILE, LAGS], f32, tag="mt", bufs=6
                        )
                        nc.vector.tensor_add(mt[:], norm_c0[g][:], nt[:])
                        mf = m0 + 5 * g
                        nc.gpsimd.dma_start(
                            out=out[:, mf : mf + 5, :].rearrange(
                                "bf mm l -> mm bf l"
                            ),
                            in_=mt[:],
                        )

    nc.compile()
    return nc


_NC_CACHE = {}


def _get_nc(n_sb=N_SB_FULL, use_f32r=True):
    key = (n_sb, use_f32r)
    if key not in _NC_CACHE:
        _NC_CACHE[key] = build_nc(n_sb, use_f32r)
    return _NC_CACHE[key]


def make_in_maps(nerv):
    xs = nerv.reshape(B * F, T, C)
    wcos, wsin, dmat, eye = build_weights()
    return [
        {
            "x": np.ascontiguousarray(xs[BF_PER_CORE * i : BF_PER_CORE * (i + 1)]),
            "wcos": wcos,
            "wsin": wsin,
            "dmat": dmat,
            "eye": eye,
        }
        for i in range(N_CORES)
    ]


def kernel(nervegram, trace=False, use_f32r=True):
    from concourse.bass_utils import run_bass_kernel_spmd

    nerv = np.ascontiguousarray(np.asarray(nervegram, dtype=np.float32))
    assert nerv.shape == (B, F, T, C)
    in_maps = make_in_maps(nerv)
    nc = _get_nc(use_f32r=use_f32r)
    res = run_bass_kernel_spmd(nc, in_maps, list(range(N_CORES)), trace=trace)
    full = np.concatenate([res.results[i]["out"] for i in range(N_CORES)], axis=0)
    out = full.reshape(B, F, NUM_FRAME, LAGS)
    if trace:
        return out, res
    return out
